# revision 1
# baseline (speedup 1.0000x reference)
"""Trainium2 Bass kernel for nn_Attention2D (B=8, C=256, H=W=32, 8 heads, d=32).

Strategy: data-parallel over batch, one batch element per NeuronCore (8 cores).

Per-core pipeline (n = H*W = 1024 tokens, head dim d = 32):
  phase 0: load x [256,1024] fp32 -> bf16; load host-prepped weights.
  qkv:     q = (scale*w_q) @ x, k = w_k @ x   ([256,1024] head-major, bf16)
           vT = x^T @ w_v^T                   (8x [128,256] bf16 j-chunks)
  sim^T:   per (head, j-chunk): matmul(lhsT=k slice [32,128], rhs=q slice
           [32,512]) -> PSUM ring tiles [128,1536]; 4 heads run concurrently
           via row groups (K=32 packing). Softmax max-subtraction is skipped
           (logits ~N(0,0.8), measured max |sim| = 4.8; exp is safe in fp32).
  exp:     ACT engine Exp over PSUM ring tiles -> bf16 SBUF. This is the
           kernel roofline: 8.4M exps/core at 128/cycle @ 1.2 GHz.
  AV:      per (head pair, i-half): accumulate over j-chunks
             main psum [128,512]: headA rows 0:32 (lhsT = vT slice [128,32]),
                                  headB rows 64:96 (tile_position=(0,64))
             den  psum [128,512]: all-ones lhsT -> denominator replicated on
                                  rows 0:32 (A) / 64:96 (B)
  norm:    rc = reciprocal_approx_fast(den); out_all = main * rc (DVE),
           partition-aligned; junk rows stay zero (pre-memset out_all).
  proj:    final = w_outT_padded^T @ out_all + b_out (padded weights have zero
           rows at junk positions) -> y [256,1024] fp32.
"""

import numpy as np
import ml_dtypes

B, DIM, H, W = 8, 256, 32, 32
NUM_HEADS = 8
DIM_HEAD = 256
D = DIM_HEAD // NUM_HEADS          # 32 per-head dim
N = H * W                          # 1024 tokens
SCALE = (DIM_HEAD / NUM_HEADS) ** (-0.5)
NCORES = 8

_BF16 = ml_dtypes.bfloat16

_PROGRAM = None  # compiled Bass program cache (one per process)


def build_kernel_body(tc, y_ap, x_ap, wqkvT_ap, woutT_ap, bout_ap, dbg=None):
    """Emit the per-core attention program into TileContext tc.

    DRAM tensors:
      x_ap:     [256, 1024] fp32   (one batch element, channels x tokens)
      wqkvT_ap: [256, 768]  bf16   (w_qkv^T, q-part pre-scaled by SCALE)
      woutT_ap: [512, 256]  bf16   (w_out^T padded: 64-row blocks per head,
                                    rows 0:32 real, 32:64 zero)
      bout_ap:  [256, 1]    fp32
      y_ap:     [256, 1024] fp32 out
    """
    from contextlib import ExitStack
    from concourse import mybir

    nc = tc.nc
    f32 = mybir.dt.float32
    bf16 = mybir.dt.bfloat16

    with ExitStack() as ctx:
        singles = ctx.enter_context(tc.tile_pool(name="singles", bufs=1))
        evac = ctx.enter_context(tc.tile_pool(name="evac", bufs=2))
        exp_pool = ctx.enter_context(tc.tile_pool(name="exp", bufs=36))
        rc_pool = ctx.enter_context(tc.tile_pool(name="rc", bufs=2))
        sim_psum = ctx.enter_context(tc.tile_pool(name="simp", bufs=2, space="PSUM"))
        acc_psum = ctx.enter_context(tc.tile_pool(name="accp", bufs=4, space="PSUM"))

        # ---- phase 0: loads + conversions + constant prep ----
        xb = []
        wq = []
        for c in range(2):
            t32 = singles.tile([128, N], f32, tag=f"x32_{c}")
            nc.sync.dma_start(out=t32, in_=x_ap[c * 128:(c + 1) * 128, :])
            tb = singles.tile([128, N], bf16, tag=f"xb_{c}")
            nc.gpsimd.tensor_copy(out=tb, in_=t32)
            xb.append(tb)
            tw = singles.tile([128, 768], bf16, tag=f"wq_{c}")
            nc.sync.dma_start(out=tw, in_=wqkvT_ap[c * 128:(c + 1) * 128, :])
            wq.append(tw)
        wo = []
        for t in range(4):
            tw = singles.tile([128, 256], bf16, tag=f"wo_{t}")
            nc.sync.dma_start(out=tw, in_=woutT_ap[t * 128:(t + 1) * 128, :])
            wo.append(tw)
        bias = []
        for oc in range(2):
            tb = singles.tile([128, 1], f32, tag=f"bias_{oc}")
            nc.sync.dma_start(out=tb, in_=bout_ap[oc * 128:(oc + 1) * 128, :])
            bias.append(tb)

        ones32 = singles.tile([128, 32], bf16, tag="ones32")
        nc.gpsimd.memset(ones32, 1.0)

        # out_all: final-GEMM rhs, 4 pair tiles x [128, 1024] bf16.
        # pair p = heads (2p, 2p+1): head A rows 0:32, head B rows 64:96.
        out_all = []
        for t in range(4):
            ta = singles.tile([128, N], bf16, tag=f"out_all_{t}")
            nc.gpsimd.memset(ta, 0.0)
            out_all.append(ta)

        # ---- qkv GEMM: q (o-chunks 0,1), k (o-chunks 2,3) ----
        qk = []
        for oc in range(4):
            dst = singles.tile([128, N], bf16, tag=f"qk_{oc}")
            for nh in range(2):
                ps = acc_psum.tile([128, 512], f32, tag="acc")
                for kc in range(2):
                    nc.tensor.matmul(
                        ps,
                        wq[kc][:, oc * 128:(oc + 1) * 128],
                        xb[kc][:, nh * 512:(nh + 1) * 512],
                        start=(kc == 0),
                        stop=(kc == 1),
                    )
                nc.vector.tensor_copy(out=dst[:, nh * 512:(nh + 1) * 512], in_=ps)
            qk.append(dst)
        qb = qk[0:2]
        kb = qk[2:4]

        # ---- vT GEMM: vt[jc] = x[:, jc]^T @ w_v^T  ([128,256] bf16) ----
        vt = []
        for jc in range(8):
            ps = acc_psum.tile([128, 256], f32, tag="acc")
            for kc in range(2):
                nc.tensor.matmul(
                    ps,
                    xb[kc][:, jc * 128:(jc + 1) * 128],
                    wq[kc][:, 512:768],
                    start=(kc == 0),
                    stop=(kc == 1),
                )
            dst = singles.tile([128, 256], bf16, tag=f"vt_{jc}")
            nc.vector.tensor_copy(out=dst, in_=ps)
            vt.append(dst)

        # ---- main loop: sim^T -> exp -> AV(+den) -> normalize ----
        # production unit u = ((Q*2 + ih)*8 + jc)*4 + hq, each [128, 512].
        # ring tiles hold 3 units -> one ACT exp instruction [128, 1536].
        exp_slices = {}
        state = {"psum": None, "exp": None, "units": 0}

        def flush_group():
            if state["psum"] is None:
                return
            w = state["units"] * 512
            nc.scalar.activation(
                out=state["exp"][:, 0:w],
                in_=state["psum"][:, 0:w],
                func=mybir.ActivationFunctionType.Exp,
            )
            state["psum"] = None
            state["exp"] = None
            state["units"] = 0

        def unit_index(Q, ih, jc, hq):
            return ((Q * 2 + ih) * 8 + jc) * 4 + hq

        for Q in range(2):
            for ih in range(2):
                for jc in range(8):
                    for hq in range(4):
                        u = unit_index(Q, ih, jc, hq)
                        if state["psum"] is None:
                            state["psum"] = sim_psum.tile([128, 1024], f32, tag="sim", name=f"sim_{u}")
                            state["exp"] = exp_pool.tile([128, 1024], bf16, tag="exp", name=f"exp_{u}")
                        s = state["units"]
                        tp = (96, 0) if hq == 3 else None
                        nc.tensor.matmul(
                            state["psum"][:, s * 512:(s + 1) * 512],
                            kb[Q][32 * hq:32 * (hq + 1), jc * 128:(jc + 1) * 128],
                            qb[Q][32 * hq:32 * (hq + 1), ih * 512:(ih + 1) * 512],
                            start=True,
                            stop=True,
                            tile_position=tp,
                        )
                        exp_slices[u] = (state["exp"], s)
                        state["units"] += 1
                        if state["units"] == 2:
                            flush_group()
                if (Q, ih) == (1, 1):
                    flush_group()

                # AV + normalize for pairs of this (Q, ih)
                for pq in range(2):
                    pair = 2 * Q + pq            # heads (2*pair, 2*pair+1)
                    hA, hB = 2 * pq, 2 * pq + 1  # in-quad head indices
                    mainA = acc_psum.tile([128, 512], f32, tag="acc",
                                          name=f"mA_{pair}_{ih}")
                    mainB = acc_psum.tile([128, 512], f32, tag="acc",
                                          name=f"mB_{pair}_{ih}")
                    denA = acc_psum.tile([128, 512], f32, tag="acc",
                                         name=f"dA_{pair}_{ih}")
                    denB = acc_psum.tile([128, 512], f32, tag="acc",
                                         name=f"dB_{pair}_{ih}")
                    for jc in range(8):
                        eA, sA = exp_slices[unit_index(Q, ih, jc, hA)]
                        eB, sB = exp_slices[unit_index(Q, ih, jc, hB)]
                        rhsA = eA[:, sA * 512:(sA + 1) * 512]
                        rhsB = eB[:, sB * 512:(sB + 1) * 512]
                        st, sp = (jc == 0), (jc == 7)
                        nc.tensor.matmul(
                            mainA[0:32, :], vt[jc][:, 32 * (4 * Q + hA):32 * (4 * Q + hA) + 32],
                            rhsA, start=st, stop=sp)
                        nc.tensor.matmul(
                            mainB[64:96, :], vt[jc][:, 32 * (4 * Q + hB):32 * (4 * Q + hB) + 32],
                            rhsB, start=st, stop=sp, tile_position=(0, 64))
                        nc.tensor.matmul(
                            denA[0:32, :], ones32, rhsA, start=st, stop=sp)
                        nc.tensor.matmul(
                            denB[64:96, :], ones32, rhsB, start=st, stop=sp,
                            tile_position=(0, 64))
                    # custom-DVE ops misbehave on base_partition != 0 slices;
                    # run them over the full tile (garbage rows never read).
                    rc = rc_pool.tile([128, 512], f32, tag="rc")
                    rcB = rc_pool.tile([128, 512], f32, tag="rcB")
                    nc.vector.reciprocal_approx_fast(out=rc[:, :], in_=denA[:, :])
                    nc.vector.reciprocal_approx_fast(out=rcB[:, :], in_=denB[:, :])
                    if dbg is not None and pair == 0 and ih == 0:
                        for nm, t_, lo in (("denA", denA, 0), ("denB", denB, 64),
                                           ("mainB", mainB, 64), ("rcd", rcB, 64)):
                            if nm in dbg:
                                tmp = rc_pool.tile([128, 512], f32, tag="dbgtmp",
                                                   name=f"dbg_{nm}")
                                nc.vector.tensor_copy(out=tmp[lo:lo + 32, :],
                                                      in_=t_[lo:lo + 32, :])
                                nc.sync.dma_start(out=dbg[nm],
                                                  in_=tmp[lo:lo + 32, :])
                    dst = out_all[pair]
                    nc.vector.tensor_mul(
                        out=dst[0:32, ih * 512:(ih + 1) * 512],
                        in0=mainA[0:32, :], in1=rc[0:32, :])
                    nc.vector.tensor_mul(
                        out=dst[64:96, ih * 512:(ih + 1) * 512],
                        in0=mainB[64:96, :], in1=rcB[64:96, :])

        if dbg is not None:
            for nm, tile_ in (("qb0", qb[0]), ("qb1", qb[1]), ("kb0", kb[0]),
                              ("kb1", kb[1]), ("vt0", vt[0]), ("vt7", vt[7]),
                              ("oa0", out_all[0]), ("oa1", out_all[1]),
                              ("oa2", out_all[2]), ("oa3", out_all[3])):
                if nm in dbg:
                    nc.sync.dma_start(out=dbg[nm], in_=tile_)
            if "exp0" in dbg:
                et, s = exp_slices[unit_index(0, 0, 0, 0)]
                nc.sync.dma_start(out=dbg["exp0"], in_=et[:, s * 512:(s + 1) * 512])
            if "exp5" in dbg:
                et, s = exp_slices[unit_index(0, 0, 1, 1)]
                nc.sync.dma_start(out=dbg["exp5"], in_=et[:, s * 512:(s + 1) * 512])

        # ---- final projection + bias ----
        for oc in range(2):
            for nh in range(2):
                ps = acc_psum.tile([128, 512], f32, tag="acc")
                for t in range(4):
                    nc.tensor.matmul(
                        ps,
                        wo[t][:, oc * 128:(oc + 1) * 128],
                        out_all[t][:, nh * 512:(nh + 1) * 512],
                        start=(t == 0),
                        stop=(t == 3),
                    )
                ys = evac.tile([128, 512], f32, tag="y")
                nc.vector.tensor_scalar_add(out=ys, in0=ps, scalar1=bias[oc])
                nc.sync.dma_start(
                    out=y_ap[oc * 128:(oc + 1) * 128, nh * 512:(nh + 1) * 512],
                    in_=ys,
                )


def _prep_weights(w_qkv, w_out, b_out):
    """Host-side weight preparation (numpy)."""
    wq = w_qkv.astype(np.float32).copy()
    wq[0:DIM_HEAD] *= SCALE                      # fold softmax scale into w_q
    wqkvT = np.ascontiguousarray(wq.T).astype(_BF16)          # [256, 768]

    w_outT = np.ascontiguousarray(w_out.astype(np.float32).T)  # [hd, o]
    pad = np.zeros((8, 64, DIM), dtype=np.float32)
    for h in range(NUM_HEADS):
        pad[h, 0:D, :] = w_outT[h * D:(h + 1) * D, :]
    woutT = pad.reshape(512, DIM).astype(_BF16)               # [512, 256]

    bout = b_out.astype(np.float32).reshape(DIM, 1)           # [256, 1]
    return wqkvT, woutT, bout


def _strip_redundant_pe_waits(nc):
    """Drop transitively-implied sem waits from PE instructions.

    Walrus allows only one sync-wait command on a Matmult. Tile's semaphore
    pass is not transitively minimal: the first matmul writing a recycled
    PSUM slot waits both on the Activation exp that freed the slot AND on a
    PE tick that the exp itself already waited for. Strip wait W2 from a PE
    instruction when another wait W1 on it is served by an instruction that
    itself waited for W2's semaphore to reach at least W2's value.
    """
    for f in nc.m.functions:
        for blk in f.blocks:
            insts = list(blk.instructions)
            cum = {}
            served_by = {}  # (sem_name, cum_value) -> inst
            for ins in insts:
                if ins.sync_info is None:
                    continue
                for up in ins.sync_info.on_update:
                    if up.update_mode != "sem-inc":
                        continue
                    c = cum.get(up.ant_name, 0) + up.update_value
                    cum[up.ant_name] = c
                    served_by[(up.ant_name, c)] = ins

            def implied(w1, w2):
                # instruction completing w1 (cum hits >= w1.value first time)
                for v in range(w1.wait_value, w1.wait_value + 16):
                    srv = served_by.get((w1.ant_name, v))
                    if srv is not None:
                        break
                else:
                    return False
                srv_si = srv.sync_info
                if srv_si is None:
                    return False
                for w in srv_si.on_wait:
                    if (w.ant_name == w2.ant_name
                            and w.wait_mode == "sem-ge-imm"
                            and w.wait_value >= w2.wait_value):
                        return True
                return False

            for ins in insts:
                if str(ins.engine) not in ("EngineType.PE", "PE"):
                    continue
                si = ins.sync_info
                if si is None:
                    continue
                waits = list(si.on_wait)
                while len(waits) > 1:
                    drop = None
                    for w2 in waits:
                        if w2.wait_mode != "sem-ge-imm":
                            continue
                        for w1 in waits:
                            if w1 is w2 or w1.wait_mode != "sem-ge-imm":
                                continue
                            if implied(w1, w2):
                                drop = w2
                                break
                        if drop is not None:
                            break
                    if drop is None:
                        # Move a non-Activation wait onto the server of the
                        # first other wait: the server completes only after
                        # the moved condition, so the original ordering is
                        # preserved while this instruction keeps one wait.
                        w1 = next((w for w in waits
                                   if w.ant_name.startswith("Activation")), None)
                        w2 = next((w for w in waits if w is not w1), None)
                        if w1 is None or w2 is None:
                            break
                        srv = None
                        for v in range(w1.wait_value, w1.wait_value + 16):
                            srv = served_by.get((w1.ant_name, v))
                            if srv is not None:
                                break
                        if srv is None or srv.sync_info is None:
                            break
                        srv.sync_info.on_wait = list(srv.sync_info.on_wait) + [w2]
                        drop = w2
                    waits = [w for w in waits if w is not drop]
                if len(waits) != len(si.on_wait):
                    si.on_wait = waits
                if len(waits) > 1:
                    print(f"WARNING: {ins.name} still has {len(waits)} waits")


def _build_program():
    global _PROGRAM
    if _PROGRAM is not None:
        return _PROGRAM
    import concourse.tile as tile
    from concourse import bacc, mybir

    nc = bacc.Bacc("TRN2", target_bir_lowering=False, debug=False,
                   num_devices=NCORES)
    x_ap = nc.dram_tensor("x", [DIM, N], mybir.dt.float32,
                          kind="ExternalInput").ap()
    wqkvT_ap = nc.dram_tensor("wqkvT", [DIM, 3 * DIM_HEAD], mybir.dt.bfloat16,
                              kind="ExternalInput").ap()
    woutT_ap = nc.dram_tensor("woutT", [512, DIM], mybir.dt.bfloat16,
                              kind="ExternalInput").ap()
    bout_ap = nc.dram_tensor("bout", [DIM, 1], mybir.dt.float32,
                             kind="ExternalInput").ap()
    y_ap = nc.dram_tensor("y", [DIM, N], mybir.dt.float32,
                          kind="ExternalOutput").ap()
    with tile.TileContext(nc) as tc:
        build_kernel_body(tc, y_ap, x_ap, wqkvT_ap, woutT_ap, bout_ap)
    nc.compile()
    _PROGRAM = nc
    return nc


def kernel(x, w_qkv, w_out, b_out, trace=False):
    """Full-input entry point: shard over batch, run on 8 cores, gather."""
    from concourse import bass_utils

    nc = _build_program()
    wqkvT, woutT, bout = _prep_weights(w_qkv, w_out, b_out)
    in_maps = []
    for b in range(B):
        in_maps.append({
            "x": np.ascontiguousarray(
                np.asarray(x[b], dtype=np.float32).reshape(DIM, N)),
            "wqkvT": wqkvT,
            "woutT": woutT,
            "bout": bout,
        })
    res = bass_utils.run_bass_kernel_spmd(
        nc, in_maps, core_ids=list(range(NCORES)), trace=trace)
    y = np.stack([res.results[b]["y"].reshape(DIM, H, W) for b in range(B)])
    kernel.last_results = res
    return y



# revision 10
# speedup vs baseline: 1.3238x; 1.3238x over previous
"""Trainium2 Bass kernel for nn_Attention2D (B=8, C=256, H=W=32, 8 heads, d=32).

Strategy: data-parallel over batch, one batch element per NeuronCore (8 cores).

Per-core pipeline (n = H*W = 1024 tokens, head dim d = 32):
  phase 0: load x [256,1024] fp32 -> bf16 (one cast on DVE, one on ACT);
           load host-prepped weights.
  qkv:     k = w_k @ x, q = (scale*w_q) @ x  ([256,1024] head-major, bf16,
           quad-0 chunks first so sim can start early)
  vpack:   vt[jc] = x[:, jc]^T @ w_v^T packed per head as
           [v(16)|ones(16)|v(16)|ones(16)] -> [128, 8*64] bf16. The ones
           columns make the AV matmul emit the softmax denominator for free.
  sim^T:   per (head, j-chunk): matmul(lhsT=k slice [32,128], rhs=q slice
           [32,512]) -> PSUM ring tiles [128,1536] (3 units); 4 heads
           coreside via row quadrants (K=32 packing).
  exp:     ACT Exp over the 3-unit PSUM tiles -> bf16 SBUF (max-subtraction
           skipped: logits ~N(0,0.8), measured max |sim| < 5; exp safe).
  AV+den:  per (pair, ih, jc): 2 matmuls (head A rows 0:64 at tile_position
           (0,0), head B rows 64:128 at (0,64)); lhsT [128,64] =
           [v(16)|ones(16)|v(16)|ones(16)] so every 32-row quadrant holds
           16 out rows + 16 denominator rows. AV is interleaved into the
           sim loop (lag 3 j-chunks) to fill PE gaps while ACT paces the
           sim ring.
  norm:    rc = reciprocal_approx_fast(acc) (full tile); stream_shuffle
           aligns 1/den onto the out rows; one full-tile multiply writes
           out_all. Junk rows become den/den ~ 1.0 and are killed by the
           zero rows of the padded projection weights.
  proj:    final = w_outT_padded^T @ out_all + b_out -> y [256,1024] fp32.
"""

import numpy as np
import ml_dtypes

B, DIM, H, W = 8, 256, 32, 32
NUM_HEADS = 8
DIM_HEAD = 256
D = DIM_HEAD // NUM_HEADS          # 32 per-head dim
N = H * W                          # 1024 tokens
SCALE = (DIM_HEAD / NUM_HEADS) ** (-0.5)
NCORES = 8

_BF16 = ml_dtypes.bfloat16

_PROGRAM = None  # compiled Bass program cache (one per process)

# stream_shuffle operates within each 32-partition quadrant (same mask for
# all quadrants). AV lhsT is interleaved [v(16)|ones(16)|v(16)|ones(16)] per
# head, so every quadrant is [out rows 0:16 | den rows 16:32]; the mask pulls
# each quadrant's 1/den rows onto its out rows (and keeps them at 16:32, so
# junk rows become den/den ~ 1).
_RC_SHUF = [16 + i for i in range(16)] + [16 + i for i in range(16)]


def build_kernel_body(tc, y_ap, x_ap, wqkvT_ap, woutT_ap, bout_ap):
    """Emit the per-core attention program into TileContext tc.

    DRAM tensors:
      x_ap:     [256, 1024] fp32   (one batch element, channels x tokens)
      wqkvT_ap: [256, 768]  bf16   (w_qkv^T, q-part pre-scaled by SCALE)
      woutT_ap: [512, 256]  bf16   (w_out^T padded: 64-row blocks per head,
                                    interleaved [w(16)|0(16)|w(16)|0(16)])
      bout_ap:  [256, 1]    fp32
      y_ap:     [256, 1024] fp32 out
    """
    from contextlib import ExitStack
    from concourse import mybir

    nc = tc.nc
    f32 = mybir.dt.float32
    bf16 = mybir.dt.bfloat16

    with ExitStack() as ctx:
        singles = ctx.enter_context(tc.tile_pool(name="singles", bufs=1))
        evac = ctx.enter_context(tc.tile_pool(name="evac", bufs=2))
        exp_pool = ctx.enter_context(tc.tile_pool(name="exp", bufs=12))
        rc_pool = ctx.enter_context(tc.tile_pool(name="rc", bufs=3))
        sim_psum = ctx.enter_context(tc.tile_pool(name="simp", bufs=2, space="PSUM"))
        acc_psum = ctx.enter_context(tc.tile_pool(name="accp", bufs=2, space="PSUM"))

        # ---- phase 0: loads + conversions + constant prep ----
        x32 = []
        wq = []
        for c in range(2):
            t32 = singles.tile([128, N], f32, tag=f"x32_{c}")
            nc.sync.dma_start(out=t32, in_=x_ap[c * 128:(c + 1) * 128, :])
            x32.append(t32)
            tw = singles.tile([128, 768], bf16, tag=f"wq_{c}")
            nc.sync.dma_start(out=tw, in_=wqkvT_ap[c * 128:(c + 1) * 128, :])
            wq.append(tw)
        wo = []
        for t in range(4):
            tw = singles.tile([128, 256], bf16, tag=f"wo_{t}")
            nc.sync.dma_start(out=tw, in_=woutT_ap[t * 128:(t + 1) * 128, :])
            wo.append(tw)
        bias = []
        for oc in range(2):
            tb = singles.tile([128, 1], f32, tag=f"bias_{oc}")
            nc.sync.dma_start(out=tb, in_=bout_ap[oc * 128:(oc + 1) * 128, :])
            bias.append(tb)

        # fp32 -> bf16 casts: one on DVE, one on ACT (both idle at start)
        xb = []
        for c in range(2):
            tb = singles.tile([128, N], bf16, tag=f"xb_{c}")
            if c == 0:
                nc.vector.tensor_copy(out=tb, in_=x32[c])
            else:
                nc.scalar.activation(out=tb, in_=x32[c],
                                     func=mybir.ActivationFunctionType.Copy)
            xb.append(tb)

        # vpack tiles: per jc, [128, 8*64] bf16. memset 1.0; v cols written by
        # strided copies from the vt GEMM. Column block for head h:
        # [v dims 0:16 | ones x16 | v dims 16:32 | ones x16].
        vpack = []
        for jc in range(8):
            tv = singles.tile([128, 512], bf16, tag=f"vpack_{jc}")
            nc.gpsimd.memset(tv, 1.0)
            vpack.append(tv)

        # out_all: final-GEMM rhs, 4 pair tiles x [128, 1024] bf16.
        # pair p = heads (2p, 2p+1): head A rows 0:64, head B rows 64:128,
        # each 64-block interleaved [out(16)|junk(16)|out(16)|junk(16)];
        # junk rows ~1.0 (den * 1/den) are killed by woutT's zero rows.
        out_all = []
        for t in range(4):
            ta = singles.tile([128, N], bf16, tag=f"out_all_{t}")
            out_all.append(ta)

        # ---- qkv GEMM helper: one o-chunk (q: oc 0,1; k: oc 2,3) ----
        qk = [None] * 4

        def emit_qkv(oc):
            dst = singles.tile([128, N], bf16, tag=f"qk_{oc}")
            for nh in range(2):
                ps = acc_psum.tile([128, 512], f32, tag="acc")
                for kc in range(2):
                    nc.tensor.matmul(
                        ps,
                        wq[kc][:, oc * 128:(oc + 1) * 128],
                        xb[kc][:, nh * 512:(nh + 1) * 512],
                        start=(kc == 0),
                        stop=(kc == 1),
                    )
                nc.vector.tensor_copy(out=dst[:, nh * 512:(nh + 1) * 512], in_=ps)
            qk[oc] = dst

        # ---- vT GEMM + packing: vt[jc] = x[:, jc]^T @ w_v^T ----
        def emit_vt(jc):
            ps = acc_psum.tile([128, 256], f32, tag="acc")
            for kc in range(2):
                nc.tensor.matmul(
                    ps,
                    xb[kc][:, jc * 128:(jc + 1) * 128],
                    wq[kc][:, 512:768],
                    start=(kc == 0),
                    stop=(kc == 1),
                )
            pv = ps[:, :].rearrange("p (h c) -> p h c", c=32)
            vv = vpack[jc][:, :].rearrange("p (h c) -> p h c", c=64)
            nc.vector.tensor_copy(out=vv[:, :, 0:16], in_=pv[:, :, 0:16])
            nc.vector.tensor_copy(out=vv[:, :, 32:48], in_=pv[:, :, 16:32])

        # quad-0 weights first so sim(Q=0) can start early
        emit_qkv(2)   # k heads 0-3
        emit_qkv(0)   # q heads 0-3
        for jc in range(8):
            emit_vt(jc)

        # ---- main loop: sim^T -> exp -> AV(+den) -> normalize ----
        # group g = (Q, ih); 32 units of [128,512] per group; exp tiles hold
        # 3 units (final tile of a group holds 2).
        for Q in range(2):
            for ih in range(2):
                exp_slices = {}      # local unit u -> (exp_tile, slot)
                cur = {"psum": None, "exp": None, "units": 0}
                acc_tiles = {}

                def flush(g_tag):
                    if cur["psum"] is None:
                        return
                    w = cur["units"] * 512
                    nc.scalar.activation(
                        out=cur["exp"][:, 0:w],
                        in_=cur["psum"][:, 0:w],
                        func=mybir.ActivationFunctionType.Exp,
                    )
                    cur["psum"] = None
                    cur["exp"] = None
                    cur["units"] = 0

                def emit_av(jc):
                    # 2 pairs x (A, B) accumulation into acc tiles
                    for pq in range(2):
                        if jc == 0:
                            acc_tiles[pq] = acc_psum.tile(
                                [128, 512], f32, tag="acc",
                                name=f"av_{Q}_{ih}_{pq}")
                        at = acc_tiles[pq]
                        for ab in range(2):
                            hq = 2 * pq + ab
                            hg = 4 * Q + hq
                            et, s = exp_slices[jc * 4 + hq]
                            nc.tensor.matmul(
                                at[64 * ab:64 * ab + 64, :],
                                vpack[jc][:, 64 * hg:64 * hg + 64],
                                et[:, s * 512:(s + 1) * 512],
                                start=(jc == 0),
                                stop=(jc == 7),
                                tile_position=(0, 64 * ab),
                                skip_group_check=True,
                            )

                for jc in range(8):
                    for hq in range(4):
                        u = jc * 4 + hq
                        if cur["psum"] is None:
                            cur["psum"] = sim_psum.tile(
                                [128, 1536], f32, tag="sim",
                                name=f"sim_{Q}_{ih}_{u}")
                            cur["exp"] = exp_pool.tile(
                                [128, 1536], bf16, tag="exp",
                                name=f"exp_{Q}_{ih}_{u}")
                        s = cur["units"]
                        nc.tensor.matmul(
                            cur["psum"][:, s * 512:(s + 1) * 512],
                            qk[2 + Q][32 * hq:32 * (hq + 1),
                                      jc * 128:(jc + 1) * 128],
                            qk[Q][32 * hq:32 * (hq + 1),
                                  ih * 512:(ih + 1) * 512],
                            start=True,
                            stop=True,
                            tile_position=(32 * hq, 0),
                        )
                        exp_slices[u] = (cur["exp"], s)
                        cur["units"] += 1
                        if cur["units"] == 3 or u == 31:
                            flush((Q, ih))
                    if jc >= 3:
                        emit_av(jc - 3)
                for jc in range(5, 8):
                    emit_av(jc)

                # second-quad qkv weights: hide behind group (0,1)'s tail
                if (Q, ih) == (0, 1):
                    emit_qkv(3)
                    emit_qkv(1)

                # normalize both pairs of this (Q, ih)
                for pq in range(2):
                    pair = 2 * Q + pq
                    at = acc_tiles[pq]
                    rc = rc_pool.tile([128, 512], f32, tag="rc",
                                      name=f"rc_{pair}_{ih}")
                    nc.vector.reciprocal_approx_fast(out=rc, in_=at[:, :])
                    rca = rc_pool.tile([128, 512], f32, tag="rca",
                                       name=f"rca_{pair}_{ih}")
                    nc.vector.stream_shuffle(out=rca, in_=rc, mask=_RC_SHUF)
                    nc.vector.tensor_mul(
                        out=out_all[pair][:, ih * 512:(ih + 1) * 512],
                        in0=at[:, :], in1=rca)

        # ---- final projection + bias ----
        for oc in range(2):
            for nh in range(2):
                ps = acc_psum.tile([128, 512], f32, tag="acc")
                for t in range(4):
                    nc.tensor.matmul(
                        ps,
                        wo[t][:, oc * 128:(oc + 1) * 128],
                        out_all[t][:, nh * 512:(nh + 1) * 512],
                        start=(t == 0),
                        stop=(t == 3),
                    )
                ys = evac.tile([128, 512], f32, tag="y")
                nc.vector.tensor_scalar_add(out=ys, in0=ps, scalar1=bias[oc])
                nc.sync.dma_start(
                    out=y_ap[oc * 128:(oc + 1) * 128, nh * 512:(nh + 1) * 512],
                    in_=ys,
                )


def _prep_weights(w_qkv, w_out, b_out):
    """Host-side weight preparation (numpy)."""
    wq = w_qkv.astype(np.float32).copy()
    wq[0:DIM_HEAD] *= SCALE                      # fold softmax scale into w_q
    wqkvT = np.ascontiguousarray(wq.T).astype(_BF16)          # [256, 768]

    w_outT = np.ascontiguousarray(w_out.astype(np.float32).T)  # [hd, o]
    # interleaved pad matching the AV lhsT layout: per head-block of 64 rows,
    # [w dims 0:16 | zeros x16 | w dims 16:32 | zeros x16]
    pad = np.zeros((8, 4, 16, DIM), dtype=np.float32)
    for h in range(NUM_HEADS):
        pad[h, 0, :, :] = w_outT[h * D:h * D + 16, :]
        pad[h, 2, :, :] = w_outT[h * D + 16:h * D + 32, :]
    woutT = pad.reshape(512, DIM).astype(_BF16)               # [512, 256]

    bout = b_out.astype(np.float32).reshape(DIM, 1)           # [256, 1]
    return wqkvT, woutT, bout


def _build_program():
    global _PROGRAM
    if _PROGRAM is not None:
        return _PROGRAM
    import concourse.tile as tile
    from concourse import bacc, mybir

    nc = bacc.Bacc("TRN2", target_bir_lowering=False, debug=False,
                   num_devices=NCORES)
    x_ap = nc.dram_tensor("x", [DIM, N], mybir.dt.float32,
                          kind="ExternalInput").ap()
    wqkvT_ap = nc.dram_tensor("wqkvT", [DIM, 3 * DIM_HEAD], mybir.dt.bfloat16,
                              kind="ExternalInput").ap()
    woutT_ap = nc.dram_tensor("woutT", [512, DIM], mybir.dt.bfloat16,
                              kind="ExternalInput").ap()
    bout_ap = nc.dram_tensor("bout", [DIM, 1], mybir.dt.float32,
                             kind="ExternalInput").ap()
    y_ap = nc.dram_tensor("y", [DIM, N], mybir.dt.float32,
                          kind="ExternalOutput").ap()
    with tile.TileContext(nc) as tc:
        build_kernel_body(tc, y_ap, x_ap, wqkvT_ap, woutT_ap, bout_ap)
    nc.compile()
    _PROGRAM = nc
    return nc


def kernel(x, w_qkv, w_out, b_out, trace=False):
    """Full-input entry point: shard over batch, run on 8 cores, gather."""
    from concourse import bass_utils

    nc = _build_program()
    wqkvT, woutT, bout = _prep_weights(w_qkv, w_out, b_out)
    in_maps = []
    for b in range(B):
        in_maps.append({
            "x": np.ascontiguousarray(
                np.asarray(x[b], dtype=np.float32).reshape(DIM, N)),
            "wqkvT": wqkvT,
            "woutT": woutT,
            "bout": bout,
        })
    res = bass_utils.run_bass_kernel_spmd(
        nc, in_maps, core_ids=list(range(NCORES)), trace=trace)
    y = np.stack([res.results[b]["y"].reshape(DIM, H, W) for b in range(B)])
    kernel.last_results = res
    return y


# revision 19
# speedup vs baseline: 1.4649x; 1.1066x over previous
"""Trainium2 Bass kernel for nn_Attention2D (B=8, C=256, H=W=32, 8 heads, d=32).

Strategy: data-parallel over batch, one batch element per NeuronCore (8 cores).

Per-core pipeline (n = H*W = 1024 tokens, head dim d = 32):
  phase 0: load x [256,1024] fp32 -> bf16 (one cast on DVE, one on ACT);
           load host-prepped weights.
  qkv:     k = w_k @ x, q = (scale*w_q) @ x  ([256,1024] head-major, bf16,
           quad-0 chunks first so sim can start early)
  vpack:   vt[jc] = x[:, jc]^T @ w_v^T packed per head as
           [v(16)|ones(16)|v(16)|ones(16)] -> [128, 8*64] bf16. The ones
           columns make the AV matmul emit the softmax denominator for free.
  sim^T:   per (head, j-chunk): matmul(lhsT=k slice [32,128], rhs=q slice
           [32,512]) -> PSUM ring tiles [128,1536] (3 units); 4 heads
           coreside via row quadrants (K=32 packing).
  exp:     ACT Exp over the 3-unit PSUM tiles -> bf16 SBUF (max-subtraction
           skipped: logits ~N(0,0.8), measured max |sim| < 5; exp safe).
  AV+den:  per (pair, ih, jc): 2 matmuls (head A rows 0:64 at tile_position
           (0,0), head B rows 64:128 at (0,64)); lhsT [128,64] =
           [v(16)|ones(16)|v(16)|ones(16)] so every 32-row quadrant holds
           16 out rows + 16 denominator rows. AV is interleaved into the
           sim loop (lag 3 j-chunks) to fill PE gaps while ACT paces the
           sim ring.
  norm:    rc = reciprocal_approx_fast(acc) (full tile); stream_shuffle
           aligns 1/den onto the out rows; one full-tile multiply writes
           out_all. Junk rows become den/den ~ 1.0 and are killed by the
           zero rows of the padded projection weights.
  proj:    final = w_outT_padded^T @ out_all + b_out -> y [256,1024] fp32.
"""

import numpy as np
import ml_dtypes

B, DIM, H, W = 8, 256, 32, 32
NUM_HEADS = 8
DIM_HEAD = 256
D = DIM_HEAD // NUM_HEADS          # 32 per-head dim
N = H * W                          # 1024 tokens
SCALE = (DIM_HEAD / NUM_HEADS) ** (-0.5)
NCORES = 8

_BF16 = ml_dtypes.bfloat16

_PROGRAM = None  # compiled Bass program cache (one per process)

# DVE exp offload: logits are pre-scaled by 1/EXPN (folded into w_q on the
# host); the ACT engine recovers exp(x) via its free affine (scale=EXPN) and
# the DVE computes exp(x) ~ (1 + x/EXPN)^EXPN with EXPN=128 -- a 1(add) +
# 7(squaring) chain that exactly fills the 8-stage v3 DVE pipeline.
EXPN = 128.0
_EXP_OP = None


def _register_exp128():
    """Register the custom-DVE op EXP128_ANT: out = (1 + in0)^128."""
    global _EXP_OP
    if _EXP_OP is not None:
        return _EXP_OP
    import concourse.dve_ops as dve_ops_mod
    from concourse.dve_spec import Spec, Src0, One, lower
    from concourse.dve_uop import DveOpSpec
    from concourse.dve_ops import DveOp

    y = Src0 + One
    for _ in range(7):
        y = y * y

    def _ref(in0, in1, c0, c1, c2):
        y = 1.0 + in0
        for _ in range(7):
            y = y * y
        return y

    spec = Spec(body=y, reference=_ref)
    name = "EXP128_ANT"
    if name in dve_ops_mod._SUB_OPCODE_FOR_NAME:
        _EXP_OP = next(op for op in dve_ops_mod.OPS if op.name == name)
        return _EXP_OP
    row = dve_ops_mod._CUSTOM_DVE_ROW_BASE + len(dve_ops_mod.OPS)
    dve_ops_mod._SUB_OPCODE_FOR_NAME[name] = row
    shas = {}
    for ver in ("v3", "v4"):
        uops = lower(spec, ver=ver)
        shas[ver] = DveOpSpec(name=name, opcode=row, uops=uops,
                              rd1_en=False).sha(ver)
    op = DveOp(name, spec, subdim=False, uops_sha=shas)
    dve_ops_mod.OPS.append(op)
    dve_ops_mod.CUSTOM_DVE_SPECS[name] = spec
    _EXP_OP = op
    return op

# stream_shuffle operates within each 32-partition quadrant (same mask for
# all quadrants). AV lhsT is interleaved [v(16)|ones(16)|v(16)|ones(16)] per
# head, so every quadrant is [out rows 0:16 | den rows 16:32]; the mask pulls
# each quadrant's 1/den rows onto its out rows (and keeps them at 16:32, so
# junk rows become den/den ~ 1).
_RC_SHUF = [16 + i for i in range(16)] + [16 + i for i in range(16)]


def build_kernel_body(tc, y_ap, x_ap, wqkvT_ap, woutT_ap, bout_ap):
    """Emit the per-core attention program into TileContext tc.

    DRAM tensors:
      x_ap:     [256, 1024] bf16   (one batch element, channels x tokens,
                                    host-cast to bf16)
      wqkvT_ap: [256, 768]  bf16   (w_qkv^T, q-part pre-scaled by SCALE)
      woutT_ap: [512, 256]  bf16   (w_out^T padded: 64-row blocks per head,
                                    interleaved [w(16)|0(16)|w(16)|0(16)])
      bout_ap:  [256, 1]    fp32
      y_ap:     [256, 1024] fp32 out
    """
    from contextlib import ExitStack
    from concourse import mybir

    nc = tc.nc
    f32 = mybir.dt.float32
    bf16 = mybir.dt.bfloat16

    with ExitStack() as ctx:
        singles = ctx.enter_context(tc.tile_pool(name="singles", bufs=1))
        evac = ctx.enter_context(tc.tile_pool(name="evac", bufs=2))
        exp_pool = ctx.enter_context(tc.tile_pool(name="exp", bufs=12))
        rc_pool = ctx.enter_context(tc.tile_pool(name="rc", bufs=3))
        sim_psum = ctx.enter_context(tc.tile_pool(name="simp", bufs=2, space="PSUM"))
        acc_psum = ctx.enter_context(tc.tile_pool(name="accp", bufs=2, space="PSUM"))

        # ---- phase 0: loads + constant prep (x is host-cast to bf16) ----
        xb = []
        wq = []
        for c in range(2):
            tb = singles.tile([128, N], bf16, tag=f"xb_{c}")
            nc.sync.dma_start(out=tb, in_=x_ap[c * 128:(c + 1) * 128, :])
            xb.append(tb)
            tw = singles.tile([128, 768], bf16, tag=f"wq_{c}")
            nc.sync.dma_start(out=tw, in_=wqkvT_ap[c * 128:(c + 1) * 128, :])
            wq.append(tw)
        wo = []
        for t in range(4):
            tw = singles.tile([128, 256], bf16, tag=f"wo_{t}")
            nc.sync.dma_start(out=tw, in_=woutT_ap[t * 128:(t + 1) * 128, :])
            wo.append(tw)
        bias = []
        for oc in range(2):
            tb = singles.tile([128, 1], f32, tag=f"bias_{oc}")
            nc.sync.dma_start(out=tb, in_=bout_ap[oc * 128:(oc + 1) * 128, :])
            bias.append(tb)

        # vpack tiles: per jc, [128, 8*64] bf16. memset 1.0; v cols written by
        # strided copies from the vt GEMM. Column block for head h:
        # [v dims 0:16 | ones x16 | v dims 16:32 | ones x16].
        vpack = []
        for jc in range(8):
            tv = singles.tile([128, 512], bf16, tag=f"vpack_{jc}")
            nc.gpsimd.memset(tv, 1.0)
            vpack.append(tv)

        # out_all: final-GEMM rhs, 4 pair tiles x [128, 1024] bf16.
        # pair p = heads (2p, 2p+1): head A rows 0:64, head B rows 64:128,
        # each 64-block interleaved [out(16)|junk(16)|out(16)|junk(16)];
        # junk rows ~1.0 (den * 1/den) are killed by woutT's zero rows.
        out_all = []
        for t in range(4):
            ta = singles.tile([128, N], bf16, tag=f"out_all_{t}")
            out_all.append(ta)

        # ---- qkv GEMM helper: one o-chunk (q: oc 0,1; k: oc 2,3) ----
        qk = [None] * 4

        def emit_qkv(oc):
            dst = singles.tile([128, N], bf16, tag=f"qk_{oc}")
            for nh in range(2):
                ps = acc_psum.tile([128, 512], f32, tag="acc")
                for kc in range(2):
                    nc.tensor.matmul(
                        ps,
                        wq[kc][:, oc * 128:(oc + 1) * 128],
                        xb[kc][:, nh * 512:(nh + 1) * 512],
                        start=(kc == 0),
                        stop=(kc == 1),
                    )
                nc.vector.tensor_copy(out=dst[:, nh * 512:(nh + 1) * 512], in_=ps)
            qk[oc] = dst

        # ---- vT GEMM + packing: vt[jc] = x[:, jc]^T @ w_v^T ----
        def emit_vt(jc):
            ps = acc_psum.tile([128, 256], f32, tag="acc")
            for kc in range(2):
                nc.tensor.matmul(
                    ps,
                    xb[kc][:, jc * 128:(jc + 1) * 128],
                    wq[kc][:, 512:768],
                    start=(kc == 0),
                    stop=(kc == 1),
                )
            pv = ps[:, :].rearrange("p (h c) -> p h c", c=32)
            vv = vpack[jc][:, :].rearrange("p (h c) -> p h c", c=64)
            nc.vector.tensor_copy(out=vv[:, :, 0:16], in_=pv[:, :, 0:16])
            nc.vector.tensor_copy(out=vv[:, :, 32:48], in_=pv[:, :, 16:32])

        # quad-0 weights first so sim(Q=0) can start early; everything else
        # up front too (a mid-loop qkv serializes PE behind DVE normalize)
        emit_qkv(2)   # k heads 0-3
        emit_qkv(0)   # q heads 0-3
        for jc in range(8):
            emit_vt(jc)
        emit_qkv(3)   # k heads 4-7
        emit_qkv(1)   # q heads 4-7

        exp_op = _register_exp128()

        # ---- main loop: sim^T -> exp -> AV(+den) -> normalize ----
        # group g = (Q, ih); 32 units of [128,512] per group; exp tiles hold
        # 3 units (final tile of a group holds 2). ih-major group order so
        # both Q quads of one ih finish back to back (proj could start early).
        for Q, ih in ((0, 0), (1, 0), (0, 1), (1, 1)):
                exp_slices = {}      # local unit u -> (exp_tile, slot)
                cur = {"psum": None, "exp": None, "units": 0, "tile": 0}

                acc_tiles = {}

                def flush(g_tag):
                    if cur["psum"] is None:
                        return
                    w = cur["units"] * 512
                    # tiles 3 and 7 of each group go to the DVE via the
                    # (1+s)^128 approximation; the rest to ACT (exact exp,
                    # recovering x via the free affine scale).
                    if cur["tile"] in (3, 7):
                        nc.vector._custom_dve(
                            exp_op,
                            out=cur["exp"][:, 0:w],
                            in0=cur["psum"][:, 0:w],
                        )
                    else:
                        nc.scalar.activation(
                            out=cur["exp"][:, 0:w],
                            in_=cur["psum"][:, 0:w],
                            func=mybir.ActivationFunctionType.Exp,
                            scale=EXPN,
                        )
                    cur["psum"] = None
                    cur["exp"] = None
                    cur["units"] = 0
                    cur["tile"] += 1

                def emit_av(jc):
                    # 2 pairs x (A, B) accumulation into acc tiles
                    for pq in range(2):
                        if jc == 0:
                            acc_tiles[pq] = acc_psum.tile(
                                [128, 512], f32, tag="acc",
                                name=f"av_{Q}_{ih}_{pq}")
                        at = acc_tiles[pq]
                        for ab in range(2):
                            hq = 2 * pq + ab
                            hg = 4 * Q + hq
                            et, s = exp_slices[jc * 4 + hq]
                            nc.tensor.matmul(
                                at[64 * ab:64 * ab + 64, :],
                                vpack[jc][:, 64 * hg:64 * hg + 64],
                                et[:, s * 512:(s + 1) * 512],
                                start=(jc == 0),
                                stop=(jc == 7),
                                tile_position=(0, 64 * ab),
                                skip_group_check=True,
                            )

                for jc in range(8):
                    for hq in range(4):
                        u = jc * 4 + hq
                        if cur["psum"] is None:
                            cur["psum"] = sim_psum.tile(
                                [128, 1536], f32, tag="sim",
                                name=f"sim_{Q}_{ih}_{u}")
                            cur["exp"] = exp_pool.tile(
                                [128, 1536], bf16, tag="exp",
                                name=f"exp_{Q}_{ih}_{u}")
                        s = cur["units"]
                        nc.tensor.matmul(
                            cur["psum"][:, s * 512:(s + 1) * 512],
                            qk[2 + Q][32 * hq:32 * (hq + 1),
                                      jc * 128:(jc + 1) * 128],
                            qk[Q][32 * hq:32 * (hq + 1),
                                  ih * 512:(ih + 1) * 512],
                            start=True,
                            stop=True,
                            tile_position=(32 * hq, 0),
                        )
                        exp_slices[u] = (cur["exp"], s)
                        cur["units"] += 1
                        if cur["units"] == 3 or u == 31:
                            flush((Q, ih))
                    if jc >= 3:
                        emit_av(jc - 3)
                for jc in range(5, 8):
                    emit_av(jc)

                # normalize both pairs of this (Q, ih)
                for pq in range(2):
                    pair = 2 * Q + pq
                    at = acc_tiles[pq]
                    rc = rc_pool.tile([128, 512], f32, tag="rc",
                                      name=f"rc_{pair}_{ih}")
                    nc.vector.reciprocal_approx_fast(out=rc, in_=at[:, :])
                    rca = rc_pool.tile([128, 512], f32, tag="rca",
                                       name=f"rca_{pair}_{ih}")
                    nc.vector.stream_shuffle(out=rca, in_=rc, mask=_RC_SHUF)
                    nc.vector.tensor_mul(
                        out=out_all[pair][:, ih * 512:(ih + 1) * 512],
                        in0=at[:, :], in1=rca)

        # ---- final projection + bias ----
        for oc in range(2):
            for nh in range(2):
                ps = acc_psum.tile([128, 512], f32, tag="acc")
                for t in range(4):
                    nc.tensor.matmul(
                        ps,
                        wo[t][:, oc * 128:(oc + 1) * 128],
                        out_all[t][:, nh * 512:(nh + 1) * 512],
                        start=(t == 0),
                        stop=(t == 3),
                    )
                ys = evac.tile([128, 512], f32, tag="y")
                nc.vector.tensor_scalar_add(out=ys, in0=ps, scalar1=bias[oc])
                nc.sync.dma_start(
                    out=y_ap[oc * 128:(oc + 1) * 128, nh * 512:(nh + 1) * 512],
                    in_=ys,
                )


def _prep_weights(w_qkv, w_out, b_out):
    """Host-side weight preparation (numpy)."""
    wq = w_qkv.astype(np.float32).copy()
    # fold softmax scale AND the 1/EXPN logit pre-scale into w_q
    wq[0:DIM_HEAD] *= SCALE / EXPN
    wqkvT = np.ascontiguousarray(wq.T).astype(_BF16)          # [256, 768]

    w_outT = np.ascontiguousarray(w_out.astype(np.float32).T)  # [hd, o]
    # interleaved pad matching the AV lhsT layout: per head-block of 64 rows,
    # [w dims 0:16 | zeros x16 | w dims 16:32 | zeros x16]
    pad = np.zeros((8, 4, 16, DIM), dtype=np.float32)
    for h in range(NUM_HEADS):
        pad[h, 0, :, :] = w_outT[h * D:h * D + 16, :]
        pad[h, 2, :, :] = w_outT[h * D + 16:h * D + 32, :]
    woutT = pad.reshape(512, DIM).astype(_BF16)               # [512, 256]

    bout = b_out.astype(np.float32).reshape(DIM, 1)           # [256, 1]
    return wqkvT, woutT, bout


def _build_program():
    global _PROGRAM
    if _PROGRAM is not None:
        return _PROGRAM
    import concourse.tile as tile
    from concourse import bacc, mybir

    nc = bacc.Bacc("TRN2", target_bir_lowering=False, debug=False,
                   num_devices=NCORES)
    x_ap = nc.dram_tensor("x", [DIM, N], mybir.dt.bfloat16,
                          kind="ExternalInput").ap()
    wqkvT_ap = nc.dram_tensor("wqkvT", [DIM, 3 * DIM_HEAD], mybir.dt.bfloat16,
                              kind="ExternalInput").ap()
    woutT_ap = nc.dram_tensor("woutT", [512, DIM], mybir.dt.bfloat16,
                              kind="ExternalInput").ap()
    bout_ap = nc.dram_tensor("bout", [DIM, 1], mybir.dt.float32,
                             kind="ExternalInput").ap()
    y_ap = nc.dram_tensor("y", [DIM, N], mybir.dt.float32,
                          kind="ExternalOutput").ap()
    with tile.TileContext(nc) as tc:
        build_kernel_body(tc, y_ap, x_ap, wqkvT_ap, woutT_ap, bout_ap)
    nc.compile()
    _PROGRAM = nc
    return nc


def kernel(x, w_qkv, w_out, b_out, trace=False):
    """Full-input entry point: shard over batch, run on 8 cores, gather."""
    from concourse import bass_utils

    nc = _build_program()
    wqkvT, woutT, bout = _prep_weights(w_qkv, w_out, b_out)
    in_maps = []
    for b in range(B):
        in_maps.append({
            "x": np.ascontiguousarray(
                np.asarray(x[b], dtype=np.float32).reshape(DIM, N)
            ).astype(_BF16),
            "wqkvT": wqkvT,
            "woutT": woutT,
            "bout": bout,
        })
    res = bass_utils.run_bass_kernel_spmd(
        nc, in_maps, core_ids=list(range(NCORES)), trace=trace)
    y = np.stack([res.results[b]["y"].reshape(DIM, H, W) for b in range(B)])
    kernel.last_results = res
    return y


# revision 29
# speedup vs baseline: 1.4842x; 1.0132x over previous
"""Trainium2 Bass kernel for nn_Attention2D (B=8, C=256, H=W=32, 8 heads, d=32).

Strategy: data-parallel over batch, one batch element per NeuronCore (8 cores).

Per-core pipeline (n = H*W = 1024 tokens, head dim d = 32):
  phase 0: load x [256,1024] fp32 -> bf16 (one cast on DVE, one on ACT);
           load host-prepped weights.
  qkv:     k = w_k @ x, q = (scale*w_q) @ x  ([256,1024] head-major, bf16,
           quad-0 chunks first so sim can start early)
  vpack:   vt[jc] = x[:, jc]^T @ w_v^T packed per head as
           [v(16)|ones(16)|v(16)|ones(16)] -> [128, 8*64] bf16. The ones
           columns make the AV matmul emit the softmax denominator for free.
  sim^T:   per (head, j-chunk): matmul(lhsT=k slice [32,128], rhs=q slice
           [32,512]) -> PSUM ring tiles [128,1536] (3 units); 4 heads
           coreside via row quadrants (K=32 packing).
  exp:     ACT Exp over the 3-unit PSUM tiles -> bf16 SBUF (max-subtraction
           skipped: logits ~N(0,0.8), measured max |sim| < 5; exp safe).
  AV+den:  per (pair, ih, jc): 2 matmuls (head A rows 0:64 at tile_position
           (0,0), head B rows 64:128 at (0,64)); lhsT [128,64] =
           [v(16)|ones(16)|v(16)|ones(16)] so every 32-row quadrant holds
           16 out rows + 16 denominator rows. AV is interleaved into the
           sim loop (lag 3 j-chunks) to fill PE gaps while ACT paces the
           sim ring.
  norm:    rc = reciprocal_approx_fast(acc) (full tile); stream_shuffle
           aligns 1/den onto the out rows; one full-tile multiply writes
           out_all. Junk rows become den/den ~ 1.0 and are killed by the
           zero rows of the padded projection weights.
  proj:    final = w_outT_padded^T @ out_all + b_out -> y [256,1024] fp32.
"""

import numpy as np
import ml_dtypes

B, DIM, H, W = 8, 256, 32, 32
NUM_HEADS = 8
DIM_HEAD = 256
D = DIM_HEAD // NUM_HEADS          # 32 per-head dim
N = H * W                          # 1024 tokens
SCALE = (DIM_HEAD / NUM_HEADS) ** (-0.5)
NCORES = 8

_BF16 = ml_dtypes.bfloat16

_PROGRAM = None  # compiled Bass program cache (one per process)

# DVE exp offload: logits are pre-scaled by 1/EXPN (folded into w_q on the
# host); the ACT engine recovers exp(x) via its free affine (scale=EXPN) and
# the DVE computes exp(x) ~ (1 + x/EXPN)^EXPN with EXPN=128 -- a 1(add) +
# 7(squaring) chain that exactly fills the 8-stage v3 DVE pipeline.
EXPN = 128.0
_EXP_OP = None


def _register_exp128():
    """Register the custom-DVE op EXP128_ANT: out = (1 + in0)^128."""
    global _EXP_OP
    if _EXP_OP is not None:
        return _EXP_OP
    import concourse.dve_ops as dve_ops_mod
    from concourse.dve_spec import Spec, Src0, One, lower
    from concourse.dve_uop import DveOpSpec
    from concourse.dve_ops import DveOp

    y = Src0 + One
    for _ in range(7):
        y = y * y

    def _ref(in0, in1, c0, c1, c2):
        y = 1.0 + in0
        for _ in range(7):
            y = y * y
        return y

    spec = Spec(body=y, reference=_ref)
    name = "EXP128_ANT"
    if name in dve_ops_mod._SUB_OPCODE_FOR_NAME:
        _EXP_OP = next(op for op in dve_ops_mod.OPS if op.name == name)
        return _EXP_OP
    row = dve_ops_mod._CUSTOM_DVE_ROW_BASE + len(dve_ops_mod.OPS)
    dve_ops_mod._SUB_OPCODE_FOR_NAME[name] = row
    shas = {}
    for ver in ("v3", "v4"):
        uops = lower(spec, ver=ver)
        shas[ver] = DveOpSpec(name=name, opcode=row, uops=uops,
                              rd1_en=False).sha(ver)
    op = DveOp(name, spec, subdim=False, uops_sha=shas)
    dve_ops_mod.OPS.append(op)
    dve_ops_mod.CUSTOM_DVE_SPECS[name] = spec
    _EXP_OP = op
    return op

# stream_shuffle operates within each 32-partition quadrant (same mask for
# all quadrants). AV lhsT is interleaved [v(16)|ones(16)|v(16)|ones(16)] per
# head, so every quadrant is [out rows 0:16 | den rows 16:32]; the mask pulls
# each quadrant's 1/den rows onto its out rows (and keeps them at 16:32, so
# junk rows become den/den ~ 1).
_RC_SHUF = [16 + i for i in range(16)] + [16 + i for i in range(16)]


def build_kernel_body(tc, y_ap, x_ap, woutT_ap, bout_ap):
    """Emit the per-core attention program into TileContext tc.

    DRAM tensors:
      x_ap:     [256, 1792] bf16   (fused [x | w_qkv^T] per channel chunk:
                                    cols 0:1024 = one batch element
                                    channels x tokens (host-cast bf16),
                                    cols 1024:1792 = w_qkv^T with the q-part
                                    pre-scaled by SCALE/EXPN)
      woutT_ap: [512, 256]  bf16   (w_out^T padded: 64-row blocks per head,
                                    interleaved [w(16)|0(16)|w(16)|0(16)])
      bout_ap:  [256, 1]    fp32
      y_ap:     [256, 1024] fp32 out
    """
    from contextlib import ExitStack
    from concourse import mybir

    nc = tc.nc
    f32 = mybir.dt.float32
    bf16 = mybir.dt.bfloat16

    with ExitStack() as ctx:
        singles = ctx.enter_context(tc.tile_pool(name="singles", bufs=1))
        evac = ctx.enter_context(tc.tile_pool(name="evac", bufs=2))
        exp_pool = ctx.enter_context(tc.tile_pool(name="exp", bufs=12))
        rc_pool = ctx.enter_context(tc.tile_pool(name="rc", bufs=3))
        sim_psum = ctx.enter_context(tc.tile_pool(name="simp", bufs=2, space="PSUM"))
        acc_psum = ctx.enter_context(tc.tile_pool(name="accp", bufs=2, space="PSUM"))

        # ---- phase 0: loads + constant prep ----
        # x (host-cast bf16) and wqkvT ride in ONE fused DMA per 128-channel
        # chunk: [128, 1024 x | 768 wq] -- halves the number of critical-path
        # DMA issues at kernel start.
        xw = []
        for c in range(2):
            txw = singles.tile([128, N + 768], bf16, tag=f"xw_{c}")
            nc.sync.dma_start(out=txw, in_=x_ap[c * 128:(c + 1) * 128, :])
            xw.append(txw)

        def XB(c, lo, hi):      # x slice, channels c*128.., tokens lo:hi
            return xw[c][:, lo:hi]

        def WQ(c, lo, hi):      # wqkvT slice, o-channels lo:hi
            return xw[c][:, N + lo:N + hi]

        wo = []
        for t in range(4):
            tw = singles.tile([128, 256], bf16, tag=f"wo_{t}")
            nc.sync.dma_start(out=tw, in_=woutT_ap[t * 128:(t + 1) * 128, :])
            wo.append(tw)
        bias = []
        for oc in range(2):
            tb = singles.tile([128, 1], f32, tag=f"bias_{oc}")
            nc.sync.dma_start(out=tb, in_=bout_ap[oc * 128:(oc + 1) * 128, :])
            bias.append(tb)

        # vpack tiles: per jc, [128, 8*64] bf16. memset 1.0; v cols written by
        # strided copies from the vt GEMM. Column block for head h:
        # [v dims 0:16 | ones x16 | v dims 16:32 | ones x16].
        vpack = []
        for jc in range(8):
            tv = singles.tile([128, 512], bf16, tag=f"vpack_{jc}")
            nc.gpsimd.memset(tv, 1.0)
            vpack.append(tv)

        # out_all: final-GEMM rhs, 4 pair tiles x [128, 1024] bf16.
        # pair p = heads (2p, 2p+1): head A rows 0:64, head B rows 64:128,
        # each 64-block interleaved [out(16)|junk(16)|out(16)|junk(16)];
        # junk rows ~1.0 (den * 1/den) are killed by woutT's zero rows.
        out_all = []
        for t in range(4):
            ta = singles.tile([128, N], bf16, tag=f"out_all_{t}")
            out_all.append(ta)

        # ---- qkv GEMM helper: one o-chunk (q: oc 0,1; k: oc 2,3) ----
        qk = [None] * 4

        def emit_qkv(oc):
            dst = singles.tile([128, N], bf16, tag=f"qk_{oc}")
            for nh in range(2):
                ps = acc_psum.tile([128, 512], f32, tag="acc")
                for kc in range(2):
                    nc.tensor.matmul(
                        ps,
                        WQ(kc, oc * 128, (oc + 1) * 128),
                        XB(kc, nh * 512, (nh + 1) * 512),
                        start=(kc == 0),
                        stop=(kc == 1),
                    )
                nc.vector.tensor_copy(out=dst[:, nh * 512:(nh + 1) * 512], in_=ps)
            qk[oc] = dst

        # ---- vT GEMM + packing: vt[jc] = x[:, jc]^T @ w_v^T ----
        def emit_vt(jc):
            ps = acc_psum.tile([128, 256], f32, tag="acc")
            for kc in range(2):
                nc.tensor.matmul(
                    ps,
                    XB(kc, jc * 128, (jc + 1) * 128),
                    WQ(kc, 512, 768),
                    start=(kc == 0),
                    stop=(kc == 1),
                )
            pv = ps[:, :].rearrange("p (h c) -> p h c", c=32)
            vv = vpack[jc][:, :].rearrange("p (h c) -> p h c", c=64)
            nc.vector.tensor_copy(out=vv[:, :, 0:16], in_=pv[:, :, 0:16])
            nc.vector.tensor_copy(out=vv[:, :, 32:48], in_=pv[:, :, 16:32])

        # quad-0 weights first so sim(Q=0) starts ASAP; vt and the quad-1
        # weights are emitted INSIDE group 1 (after its first sim tiles) so
        # the ACT exp stream starts ~5us earlier. The acc-pool psum they use
        # is free until group 1's first AV allocation (at its jc=3).
        emit_qkv(2)   # k heads 0-3
        emit_qkv(0)   # q heads 0-3

        exp_op = _register_exp128()

        # projection helper: one (oc, nh) unit = 4 accumulating matmuls +
        # bias + store. nh=0 runs early (after group 2); nh=1 in the tail.
        def emit_proj(oc, nh):
            ps = acc_psum.tile([128, 512], f32, tag="acc")
            for t in range(4):
                nc.tensor.matmul(
                    ps,
                    wo[t][:, oc * 128:(oc + 1) * 128],
                    out_all[t][:, nh * 512:(nh + 1) * 512],
                    start=(t == 0),
                    stop=(t == 3),
                )
            ys = evac.tile([128, 512], f32, tag="y")
            nc.vector.tensor_scalar_add(out=ys, in0=ps, scalar1=bias[oc])
            nc.sync.dma_start(
                out=y_ap[oc * 128:(oc + 1) * 128, nh * 512:(nh + 1) * 512],
                in_=ys,
            )

        # ---- main loop: sim^T -> exp -> AV(+den) -> normalize ----
        # group g = (Q, ih); 32 units of [128,512] per group; exp tiles hold
        # 3 units (final tile of a group holds 2). ih-major group order so
        # both Q quads of one ih finish back to back (proj could start early).
        for gi, (Q, ih) in enumerate(((0, 0), (1, 0), (0, 1), (1, 1))):
                exp_slices = {}      # local unit u -> (exp_tile, slot)
                cur = {"psum": None, "exp": None, "units": 0, "tile": 0}

                acc_tiles = {}

                def flush(g_tag):
                    if cur["psum"] is None:
                        return
                    w = cur["units"] * 512
                    # tiles 3 and 7 of each group go to the DVE via the
                    # (1+s)^128 approximation; the rest to ACT (exact exp,
                    # recovering x via the free affine scale).
                    if cur["tile"] in (3, 7):
                        nc.vector._custom_dve(
                            exp_op,
                            out=cur["exp"][:, 0:w],
                            in0=cur["psum"][:, 0:w],
                        )
                    else:
                        nc.scalar.activation(
                            out=cur["exp"][:, 0:w],
                            in_=cur["psum"][:, 0:w],
                            func=mybir.ActivationFunctionType.Exp,
                            scale=EXPN,
                        )
                    cur["psum"] = None
                    cur["exp"] = None
                    cur["units"] = 0
                    cur["tile"] += 1

                def emit_av(jc):
                    # 2 pairs x (A, B) accumulation into acc tiles
                    for pq in range(2):
                        if jc == 0:
                            acc_tiles[pq] = acc_psum.tile(
                                [128, 512], f32, tag="acc",
                                name=f"av_{Q}_{ih}_{pq}")
                        at = acc_tiles[pq]
                        for ab in range(2):
                            hq = 2 * pq + ab
                            hg = 4 * Q + hq
                            et, s = exp_slices[jc * 4 + hq]
                            nc.tensor.matmul(
                                at[64 * ab:64 * ab + 64, :],
                                vpack[jc][:, 64 * hg:64 * hg + 64],
                                et[:, s * 512:(s + 1) * 512],
                                start=(jc == 0),
                                stop=(jc == 7),
                                tile_position=(0, 64 * ab),
                                skip_group_check=True,
                            )

                for jc in range(8):
                    if gi == 0 and jc == 3:
                        # PE is 2 tiles ahead of ACT here -- spend the slack
                        # on vt + quad-1 qkv weights
                        for vjc in range(8):
                            emit_vt(vjc)
                        emit_qkv(3)   # k heads 4-7
                        emit_qkv(1)   # q heads 4-7
                    for hq in range(4):
                        u = jc * 4 + hq
                        if cur["psum"] is None:
                            cur["psum"] = sim_psum.tile(
                                [128, 1536], f32, tag="sim",
                                name=f"sim_{Q}_{ih}_{u}")
                            cur["exp"] = exp_pool.tile(
                                [128, 1536], bf16, tag="exp",
                                name=f"exp_{Q}_{ih}_{u}")
                        s = cur["units"]
                        nc.tensor.matmul(
                            cur["psum"][:, s * 512:(s + 1) * 512],
                            qk[2 + Q][32 * hq:32 * (hq + 1),
                                      jc * 128:(jc + 1) * 128],
                            qk[Q][32 * hq:32 * (hq + 1),
                                  ih * 512:(ih + 1) * 512],
                            start=True,
                            stop=True,
                            tile_position=(32 * hq, 0),
                        )
                        exp_slices[u] = (cur["exp"], s)
                        cur["units"] += 1
                        if cur["units"] == 3 or u == 31:
                            flush((Q, ih))
                    if jc >= 3:
                        emit_av(jc - 3)
                for jc in range(5, 8):
                    emit_av(jc)

                # normalize both pairs of this (Q, ih)
                for pq in range(2):
                    pair = 2 * Q + pq
                    at = acc_tiles[pq]
                    rc = rc_pool.tile([128, 512], f32, tag="rc",
                                      name=f"rc_{pair}_{ih}")
                    nc.vector.reciprocal_approx_fast(out=rc, in_=at[:, :])
                    rca = rc_pool.tile([128, 512], f32, tag="rca",
                                       name=f"rca_{pair}_{ih}")
                    nc.vector.stream_shuffle(out=rca, in_=rc, mask=_RC_SHUF)
                    nc.vector.tensor_mul(
                        out=out_all[pair][:, ih * 512:(ih + 1) * 512],
                        in0=at[:, :], in1=rca)

                if gi == 1:
                    # ih=0 halves of all pairs are done -- run the nh=0
                    # projection now; its stores overlap groups 3-4.
                    emit_proj(0, 0)
                    emit_proj(1, 0)

        # ---- tail projection (nh=1) ----
        emit_proj(0, 1)
        emit_proj(1, 1)


def _prep_weights(w_qkv, w_out, b_out):
    """Host-side weight preparation (numpy)."""
    wq = w_qkv.astype(np.float32).copy()
    # fold softmax scale AND the 1/EXPN logit pre-scale into w_q
    wq[0:DIM_HEAD] *= SCALE / EXPN
    wqkvT = np.ascontiguousarray(wq.T).astype(_BF16)          # [256, 768]

    w_outT = np.ascontiguousarray(w_out.astype(np.float32).T)  # [hd, o]
    # interleaved pad matching the AV lhsT layout: per head-block of 64 rows,
    # [w dims 0:16 | zeros x16 | w dims 16:32 | zeros x16]
    pad = np.zeros((8, 4, 16, DIM), dtype=np.float32)
    for h in range(NUM_HEADS):
        pad[h, 0, :, :] = w_outT[h * D:h * D + 16, :]
        pad[h, 2, :, :] = w_outT[h * D + 16:h * D + 32, :]
    woutT = pad.reshape(512, DIM).astype(_BF16)               # [512, 256]

    bout = b_out.astype(np.float32).reshape(DIM, 1)           # [256, 1]
    return wqkvT, woutT, bout


def _build_program():
    global _PROGRAM
    if _PROGRAM is not None:
        return _PROGRAM
    import concourse.tile as tile
    from concourse import bacc, mybir

    nc = bacc.Bacc("TRN2", target_bir_lowering=False, debug=False,
                   num_devices=NCORES)
    xw_ap = nc.dram_tensor("xw", [DIM, N + 3 * DIM_HEAD], mybir.dt.bfloat16,
                           kind="ExternalInput").ap()
    woutT_ap = nc.dram_tensor("woutT", [512, DIM], mybir.dt.bfloat16,
                              kind="ExternalInput").ap()
    bout_ap = nc.dram_tensor("bout", [DIM, 1], mybir.dt.float32,
                             kind="ExternalInput").ap()
    y_ap = nc.dram_tensor("y", [DIM, N], mybir.dt.float32,
                          kind="ExternalOutput").ap()
    with tile.TileContext(nc) as tc:
        build_kernel_body(tc, y_ap, xw_ap, woutT_ap, bout_ap)
    nc.compile()
    _PROGRAM = nc
    return nc


def kernel(x, w_qkv, w_out, b_out, trace=False):
    """Full-input entry point: shard over batch, run on 8 cores, gather."""
    from concourse import bass_utils

    nc = _build_program()
    wqkvT, woutT, bout = _prep_weights(w_qkv, w_out, b_out)
    in_maps = []
    for b in range(B):
        xb16 = np.asarray(x[b], dtype=np.float32).reshape(DIM, N).astype(_BF16)
        in_maps.append({
            "xw": np.ascontiguousarray(np.concatenate([xb16, wqkvT], axis=1)),
            "woutT": woutT,
            "bout": bout,
        })
    res = bass_utils.run_bass_kernel_spmd(
        nc, in_maps, core_ids=list(range(NCORES)), trace=trace)
    y = np.stack([res.results[b]["y"].reshape(DIM, H, W) for b in range(B)])
    kernel.last_results = res
    return y
